# revision 8
# baseline (speedup 1.0000x reference)
"""Trainium2 Bass kernel for the 8-model batch-functional CNN.

Sharding: one hypernetwork model per NeuronCore (8 models / 8 cores).
Each core runs the full 7-conv + 2-fc stack for its model over all 128
images, activations resident in SBUF.

Layout: SBUF activations are [128 partitions = 4 image-groups x 32
channels, imgs-per-chunk, H+2, W+2] (zero-padded borders).  3x3 convs
are 9 PSUM-accumulated fp32r matmuls with block-diagonal [128,128]
weights and dy/dx-shifted rhs views.  conv0 uses a host-side im2col
(K=108 = 4 groups x 3ch x 9 taps) so it is one matmul per tile.
MaxPool2d(2) is two strided tensor_max ops.  FC layers run per-group
K=32 / K=128 matmuls.  Image index = g*32 + i.
"""
import sys

sys.path.insert(0, "/opt/trn_rl_repo")
import numpy as np

N_MODELS = 8
N_IMG = 128
IMG_PER_GROUP = 32   # images assigned to each of the 4 partition groups
CHUNK = 4            # images per group processed per chunk
N_CHUNKS = IMG_PER_GROUP // CHUNK


def round_fp32r(a):
    a = np.ascontiguousarray(a, dtype=np.float32)
    b = a.view(np.uint32)
    low = b & np.uint32(0xFFF)
    bit12 = (b >> np.uint32(12)) & np.uint32(1)
    up = (low > 0x800) | ((low == 0x800) & (bit12 == 1))
    out = (b & np.uint32(0xFFFFF000)) + (up.astype(np.uint32) << np.uint32(12))
    return out.view(np.float32)


def _enable_ldw_opt():
    """Turn on walrus's weight-tile cache so back-to-back matmuls with the
    same stationary operand skip the redundant LDWEIGHTS."""
    import concourse.bass_utils as bu
    if getattr(bu.run_command, "_ldw_patched", False):
        return
    orig = bu.run_command

    def run_command(cmd, *a, **kw):
        cmd = ["--enable-ldw-opt=true" if c == "--enable-ldw-opt=false" else c
               for c in cmd]
        return orig(cmd, *a, **kw)

    run_command._ldw_patched = True
    bu.run_command = run_command


def _build_program():
    import concourse.bacc as bacc
    import concourse.tile as tile
    from concourse import mybir

    _enable_ldw_opt()

    f32 = mybir.dt.float32
    f32r = mybir.dt.float32r
    Relu = mybir.ActivationFunctionType.Relu
    Ident = mybir.ActivationFunctionType.Identity

    nc = bacc.Bacc("TRN2", target_bir_lowering=False, debug=False)

    x0s_d = nc.declare_dram_parameter("x0s", [108, IMG_PER_GROUP, 32, 32], f32r, isOutput=False)
    lt0_d = nc.declare_dram_parameter("lt0", [108, 128], f32r, isOutput=False)
    lt16_d = nc.declare_dram_parameter("lt16", [128, 6, 9, 128], f32r, isOutput=False)
    lt7_d = nc.declare_dram_parameter("lt7", [128, 2, 16, 128], f32r, isOutput=False)
    lt8_d = nc.declare_dram_parameter("lt8", [128, 2, 10], f32r, isOutput=False)
    bias_d = nc.declare_dram_parameter("biases", [128, 10], f32, isOutput=False)
    out_d = nc.declare_dram_parameter("out", [10, 4, IMG_PER_GROUP], f32, isOutput=True)

    # (layer index 1..6) -> (padded H, pooled?)  H == W throughout.
    conv_h = {1: 32, 2: 32, 3: 16, 4: 16, 5: 8, 6: 8}
    pools_after = {2, 4, 6}
    # images per psum tile so that N = imgs*H*W == 512
    tile_imgs = {1: 1, 2: 1, 3: 2, 4: 2, 5: 4, 6: 4}

    with tile.TileContext(nc) as tc:
        with tc.tile_pool(name="wpool", bufs=1) as wpool, \
             tc.tile_pool(name="acts", bufs=2) as acts, \
             tc.tile_pool(name="x0pool", bufs=1) as x0pool, \
             tc.tile_pool(name="tmp", bufs=2) as tmp, \
             tc.tile_pool(name="persist", bufs=1) as persist, \
             tc.tile_pool(name="cps", bufs=4, space="PSUM") as cps, \
             tc.tile_pool(name="fps", bufs=2, space="PSUM") as fps:

            lt0 = wpool.tile([108, 128], f32r, tag="lt0")
            lt16 = wpool.tile([128, 6, 9, 128], f32r, tag="lt16")
            lt7 = wpool.tile([128, 2, 16, 128], f32r, tag="lt7")
            lt8 = wpool.tile([128, 2, 10], f32r, tag="lt8")
            bias = wpool.tile([128, 10], f32, tag="bias")
            nc.sync.dma_start(out=lt0[:], in_=lt0_d[:])
            nc.sync.dma_start(out=lt16[:], in_=lt16_d[:])
            nc.sync.dma_start(out=lt7[:], in_=lt7_d[:])
            nc.sync.dma_start(out=lt8[:], in_=lt8_d[:])
            nc.sync.dma_start(out=bias[:], in_=bias_d[:])

            # conv6+pool output, persists across chunks: [128, 32 imgs, 6, 6]
            x7 = persist.tile([128, IMG_PER_GROUP, 6, 6], f32r, tag="x7")
            nc.vector.memset(x7[:].bitcast(f32), 0.0)

            for ch in range(N_CHUNKS):
                x0c = x0pool.tile([108, CHUNK, 32, 32], f32r, tag="x0c")
                nc.sync.dma_start(
                    out=x0c[:], in_=x0s_d[:, ch * CHUNK:(ch + 1) * CHUNK, :, :])

                # --- conv0: one K=108 matmul per (img, 16-row strip) ---
                xk = acts.tile([128, CHUNK, 34, 34], f32r, tag="big")
                nc.vector.memset(xk[:, :, 0, :].bitcast(f32), 0.0)
                nc.vector.memset(xk[:, :, 33, :].bitcast(f32), 0.0)
                nc.vector.memset(xk[:, :, 1:33, 0].bitcast(f32), 0.0)
                nc.vector.memset(xk[:, :, 1:33, 33].bitcast(f32), 0.0)
                for i in range(CHUNK):
                    for s in range(2):
                        ps = cps.tile([128, 512], f32, tag="cps")
                        nc.tensor.matmul(
                            ps[:], lt0[:], x0c[:, i, 16 * s:16 * s + 16, :],
                            start=True, stop=True)
                        nc.scalar.activation(
                            xk[:, i, 1 + 16 * s:17 + 16 * s, 1:33],
                            ps[:].rearrange("p (h w) -> p h w", h=16),
                            Relu, bias=bias[:, 0:1])

                # --- conv1..conv6 ---
                for L in range(1, 7):
                    h = conv_h[L]
                    ti = tile_imgs[L]
                    pool_after = L in pools_after
                    hn = h // 2 if pool_after else h  # next-layer H
                    ntag = {32: "big2", 16: "med2", 8: "sml2"}[h] if not pool_after \
                        else {16: "med", 8: "sml", 4: "x7"}[hn]
                    if ntag == "x7":
                        xn = x7
                    else:
                        xn = acts.tile([128, CHUNK, hn + 2, hn + 2], f32r, tag=ntag)
                        nc.vector.memset(xn[:, :, 0, :].bitcast(f32), 0.0)
                        nc.vector.memset(xn[:, :, hn + 1, :].bitcast(f32), 0.0)
                        nc.vector.memset(xn[:, :, 1:hn + 1, 0].bitcast(f32), 0.0)
                        nc.vector.memset(xn[:, :, 1:hn + 1, hn + 1].bitcast(f32), 0.0)

                    strips = max(1, (h * h * ti) // 512)
                    rows = h // strips
                    nfree = ti * rows * h

                    def evict(i0, y0, ps, L=L, h=h, ti=ti, rows=rows,
                              pool_after=pool_after, xn=xn, ntag=ntag, ch=ch):
                        psv = ps[:].rearrange(
                            "p (i h w) -> p i h w", i=ti, h=rows)
                        if not pool_after:
                            nc.scalar.activation(
                                xn[:, i0:i0 + ti, 1 + y0:1 + y0 + rows, 1:1 + h],
                                psv, Relu, bias=bias[:, L:L + 1])
                            return
                        tc_t = tmp.tile([128, ti, rows, h], f32r, tag=f"tmp{h}")
                        nc.scalar.activation(
                            tc_t[:], psv, Relu, bias=bias[:, L:L + 1])
                        # maxpool 2x2: w-pairs then h-pairs
                        th = tmp.tile([128, ti, rows, h // 2], f32r,
                                      tag=f"tmph{h}")
                        t4 = tc_t[:].rearrange(
                            "p i h (w two) -> p i h w two", two=2)
                        nc.vector.tensor_max(
                            th[:], t4[:, :, :, :, 0], t4[:, :, :, :, 1])
                        t5 = th[:].rearrange(
                            "p i (h two) w -> p i h two w", two=2)
                        ibase = ch * CHUNK + i0 if ntag == "x7" else i0
                        dsty = xn[:, ibase:ibase + ti,
                                  1 + y0 // 2:1 + (y0 + rows) // 2,
                                  1:1 + h // 2]
                        nc.vector.tensor_max(
                            dsty, t5[:, :, :, 0, :], t5[:, :, :, 1, :])

                    tiles = [(it * ti, s * rows) for it in range(CHUNK // ti)
                             for s in range(strips)]
                    # groups of 4 psum banks, taps outer, so consecutive
                    # matmuls reuse the loaded weight tile (ldw-opt)
                    for g0 in range(0, len(tiles), 4):
                        grp = tiles[g0:g0 + 4]
                        pss = []
                        for _pi in range(len(grp)):
                            ps_g = cps.tile([128, nfree], f32, tag="cps")
                            pss.append(ps_g)
                        taps = [(dy, dx) for dy in (-1, 0, 1)
                                for dx in (-1, 0, 1)]
                        for t, (dy, dx) in enumerate(taps):
                            for (i0, y0), ps in zip(grp, pss):
                                rhs = xk[:, i0:i0 + ti,
                                         1 + y0 + dy:1 + y0 + dy + rows,
                                         1 + dx:1 + dx + h]
                                nc.tensor.matmul(
                                    ps[:], lt16[:, L - 1, t, :], rhs,
                                    start=(t == 0), stop=(t == 8))
                        for (i0, y0), ps in zip(grp, pss):
                            evict(i0, y0, ps)
                    xk = xn

            # --- fc7: relu(flatten) @ W7 + b7, per group, K=32 x 16 positions ---
            f7 = persist.tile([128, 2, 4, IMG_PER_GROUP], f32r, tag="f7")
            for g in range(4):
                for hh in range(2):
                    ps = fps.tile([128, IMG_PER_GROUP], f32, tag="fps")
                    t = 0
                    for y in range(4):
                        for x in range(4):
                            nc.tensor.matmul(
                                ps[:],
                                lt7[32 * g:32 * g + 32, hh, 4 * y + x, :],
                                x7[32 * g:32 * g + 32, :, 1 + y, 1 + x],
                                start=(t == 0), stop=(t == 15),
                                tile_position=(32 * g, 0))
                            t += 1
                    nc.scalar.activation(
                        f7[:, hh, g, :], ps[:], Relu, bias=bias[:, 7 + hh:8 + hh])

            # --- fc8: f7 @ W8 + b8 -> [10, g, i] ---
            outt = persist.tile([10, 4, IMG_PER_GROUP], f32, tag="outt")
            for g in range(4):
                ps8 = fps.tile([10, IMG_PER_GROUP], f32, tag="f8")
                for hh in range(2):
                    nc.tensor.matmul(ps8[:], lt8[:, hh, :], f7[:, hh, g, :],
                                     start=(hh == 0), stop=(hh == 1))
                nc.scalar.activation(outt[:, g, :], ps8[:], Ident,
                                     bias=bias[0:10, 9:10])
            nc.sync.dma_start(out=out_d[:], in_=outt[:])

    nc.finalize()
    return nc


_NC_CACHE = None


def _get_program():
    global _NC_CACHE
    if _NC_CACHE is None:
        _NC_CACHE = _build_program()
    return _NC_CACHE


def _prep_host_inputs(x, ws, bs):
    """Build per-core input maps.  ws/bs: lists of the 9 weight/bias arrays."""
    # conv0 im2col, identical for every core: [108, 32, 32, 32]
    xp = np.zeros((N_IMG, 3, 34, 34), np.float32)
    xp[:, :, 1:33, 1:33] = x
    x0s = np.empty((108, IMG_PER_GROUP, 32, 32), np.float32)
    for g in range(4):
        sl = xp[g * 32:(g + 1) * 32]
        for c in range(3):
            for ky in range(3):
                for kx in range(3):
                    x0s[27 * g + 9 * c + 3 * ky + kx] = \
                        sl[:, c, ky:ky + 32, kx:kx + 32]
    x0s = round_fp32r(x0s)

    in_maps = []
    for m in range(N_MODELS):
        lt0 = np.zeros((108, 128), np.float32)
        w0m = ws[0][m].transpose(0, 2, 1).reshape(27, 32)  # [c,o,t]->[c,t,o]
        for g in range(4):
            lt0[27 * g:27 * g + 27, 32 * g:32 * g + 32] = w0m

        lt16 = np.zeros((128, 6, 9, 128), np.float32)
        for L in range(1, 7):
            wm = ws[L][m].transpose(0, 2, 1)  # [32c, 9t, 32o]
            for g in range(4):
                lt16[32 * g:32 * g + 32, L - 1, :, 32 * g:32 * g + 32] = wm

        lt7 = np.zeros((128, 2, 16, 128), np.float32)
        for hh in range(2):
            blk = ws[7][m][:, 128 * hh:128 * (hh + 1), :].transpose(0, 2, 1)
            for g in range(4):
                lt7[32 * g:32 * g + 32, hh] = blk  # [c, yx, o]

        lt8 = np.zeros((128, 2, 10), np.float32)
        for hh in range(2):
            lt8[:, hh, :] = ws[8][m][128 * hh:128 * (hh + 1), :, 0]

        biases = np.zeros((128, 10), np.float32)
        for L in range(7):
            bL = bs[L][m][:, 0]  # [32]
            for g in range(4):
                biases[32 * g:32 * g + 32, L] = bL
        for hh in range(2):
            biases[:, 7 + hh] = bs[7][m][128 * hh:128 * (hh + 1), 0]
        biases[0:10, 9] = bs[8][m][:, 0]

        in_maps.append({
            "x0s": x0s,
            "lt0": round_fp32r(lt0),
            "lt16": round_fp32r(lt16),
            "lt7": round_fp32r(lt7),
            "lt8": round_fp32r(lt8),
            "biases": biases,
        })
    return in_maps


def kernel(x, w0, w1, w2, w3, w4, w5, w6, w7, w8,
           b0, b1, b2, b3, b4, b5, b6, b7, b8):
    from concourse.bass_utils import run_bass_kernel_spmd

    ws = [np.asarray(w, np.float32) for w in
          (w0, w1, w2, w3, w4, w5, w6, w7, w8)]
    bs = [np.asarray(b, np.float32) for b in
          (b0, b1, b2, b3, b4, b5, b6, b7, b8)]
    nc = _get_program()
    in_maps = _prep_host_inputs(np.asarray(x, np.float32), ws, bs)
    res = run_bass_kernel_spmd(nc, in_maps, list(range(N_MODELS)))
    out = np.stack([res.results[m]["out"].reshape(10, N_IMG).T
                    for m in range(N_MODELS)])
    return out.astype(np.float32)


# revision 12
# speedup vs baseline: 1.1226x; 1.1226x over previous
"""Trainium2 Bass kernel for the 8-model batch-functional CNN.

Sharding: one hypernetwork model per NeuronCore (8 models / 8 cores).
Each core runs the full 7-conv + 2-fc stack for its model over all 128
images, activations resident in SBUF.

Layout: SBUF activations are [128 partitions = 4 image-groups x 32
channels, imgs, H+2, W+2] (zero-padded borders).  3x3 convs are 9
PSUM-accumulated fp32r matmuls with block-diagonal [128,128] weights
and dy/dx-shifted rhs views; matmuls are grouped taps-outer over up to
8 PSUM banks so walrus's weight-tile cache (ldw-opt) elides redundant
LDWEIGHTS.  conv0 uses a host-side im2col (K=108).  MaxPool2d(2) is
two strided tensor_max ops.  The 8x8 layers (conv5/6) run once over
all 32 images per group for longer same-weight runs.  fc7 runs
X7-stationary on the diagonal 32x32 tile positions (4 concurrent
tiles), fc8 after a PE transpose.  Image index = g*32 + i.
"""
import sys

sys.path.insert(0, "/opt/trn_rl_repo")
import numpy as np

N_MODELS = 8
N_IMG = 128
IMG_PER_GROUP = 32   # images assigned to each of the 4 partition groups
CHUNK = 4            # images per group per chunk through conv0..conv4
N_CHUNKS = IMG_PER_GROUP // CHUNK


def round_fp32r(a):
    a = np.ascontiguousarray(a, dtype=np.float32)
    b = a.view(np.uint32)
    low = b & np.uint32(0xFFF)
    bit12 = (b >> np.uint32(12)) & np.uint32(1)
    up = (low > 0x800) | ((low == 0x800) & (bit12 == 1))
    out = (b & np.uint32(0xFFFFF000)) + (up.astype(np.uint32) << np.uint32(12))
    return out.view(np.float32)


def _enable_ldw_opt():
    """Turn on walrus's weight-tile cache so back-to-back matmuls with the
    same stationary operand skip the redundant LDWEIGHTS."""
    import concourse.bass_utils as bu
    if getattr(bu.run_command, "_ldw_patched", False):
        return
    orig = bu.run_command

    def run_command(cmd, *a, **kw):
        cmd = ["--enable-ldw-opt=true" if c == "--enable-ldw-opt=false" else c
               for c in cmd]
        return orig(cmd, *a, **kw)

    run_command._ldw_patched = True
    bu.run_command = run_command


def _build_program():
    import concourse.bacc as bacc
    import concourse.tile as tile
    from concourse import mybir

    _enable_ldw_opt()

    f32 = mybir.dt.float32
    f32r = mybir.dt.float32r
    Relu = mybir.ActivationFunctionType.Relu
    Ident = mybir.ActivationFunctionType.Identity

    nc = bacc.Bacc("TRN2", target_bir_lowering=False, debug=False)

    x0s_d = nc.declare_dram_parameter("x0s", [108, IMG_PER_GROUP, 32, 32], f32r, isOutput=False)
    lt0_d = nc.declare_dram_parameter("lt0", [108, 128], f32r, isOutput=False)
    lt16_d = nc.declare_dram_parameter("lt16", [128, 6, 9, 128], f32r, isOutput=False)
    lt7_d = nc.declare_dram_parameter("lt7", [128, 16, 256], f32r, isOutput=False)
    lt8_d = nc.declare_dram_parameter("lt8", [128, 2, 10], f32r, isOutput=False)
    b7bc_d = nc.declare_dram_parameter("b7bc", [128, 256], f32, isOutput=False)
    idt_d = nc.declare_dram_parameter("idt", [128, 128], f32r, isOutput=False)
    bias_d = nc.declare_dram_parameter("biases", [128, 10], f32, isOutput=False)
    out_d = nc.declare_dram_parameter("out", [10, N_IMG], f32, isOutput=True)

    conv_h = {1: 32, 2: 32, 3: 16, 4: 16, 5: 8, 6: 8}
    pools_after = {2, 4, 6}
    tile_imgs = {1: 1, 2: 1, 3: 2, 4: 2, 5: 4, 6: 4}
    TAPS = [(dy, dx) for dy in (-1, 0, 1) for dx in (-1, 0, 1)]

    with tile.TileContext(nc) as tc:
        with tc.tile_pool(name="wpool", bufs=1) as wpool, \
             tc.tile_pool(name="acts", bufs=1) as acts, \
             tc.tile_pool(name="x0pool", bufs=2) as x0pool, \
             tc.tile_pool(name="tmp", bufs=2) as tmp, \
             tc.tile_pool(name="persist", bufs=1) as persist, \
             tc.tile_pool(name="cps", bufs=8, space="PSUM") as cps:

            lt0 = wpool.tile([108, 128], f32r, tag="lt0")
            lt16 = wpool.tile([128, 6, 9, 128], f32r, tag="lt16")
            lt7 = wpool.tile([128, 16, 256], f32r, tag="lt7")
            lt8 = wpool.tile([128, 2, 10], f32r, tag="lt8")
            b7bc = wpool.tile([128, 256], f32, tag="b7bc")
            idt = wpool.tile([128, 128], f32r, tag="idt")
            bias = wpool.tile([128, 10], f32, tag="bias")
            nc.sync.dma_start(out=lt0[:], in_=lt0_d[:])
            nc.sync.dma_start(out=lt16[:], in_=lt16_d[:])
            nc.sync.dma_start(out=lt7[:], in_=lt7_d[:])
            nc.sync.dma_start(out=lt8[:], in_=lt8_d[:])
            nc.sync.dma_start(out=b7bc[:], in_=b7bc_d[:])
            nc.sync.dma_start(out=idt[:], in_=idt_d[:])
            nc.sync.dma_start(out=bias[:], in_=bias_d[:])

            # persistent buffers (across chunks)
            x45 = persist.tile([128, IMG_PER_GROUP, 10, 10], f32r, tag="x45")
            x56 = persist.tile([128, IMG_PER_GROUP, 10, 10], f32r, tag="x56")
            x7 = persist.tile([128, IMG_PER_GROUP, 6, 6], f32r, tag="x7")
            for t_ in (x45, x56, x7):
                nc.vector.memset(t_[:].bitcast(f32), 0.0)

            def conv_layer(L, xk, xn, tiles, ti, h, rows, glob_dst):
                """One 3x3 conv (+optional pool) on tiles [(i0_src, y0)].
                glob_dst: None -> xn indexed like xk; else offset added to
                i0 for the destination (pool target is a persistent buf)."""
                pool_after = L in pools_after
                nfree = ti * rows * h
                for g0 in range(0, len(tiles), 8):
                    grp = tiles[g0:g0 + 8]
                    pss = []
                    for _pi in range(len(grp)):
                        ps_g = cps.tile([128, nfree], f32, tag="cps")
                        pss.append(ps_g)
                    for t, (dy, dx) in enumerate(TAPS):
                        for (i0, y0), ps in zip(grp, pss):
                            rhs = xk[:, i0:i0 + ti,
                                     1 + y0 + dy:1 + y0 + dy + rows,
                                     1 + dx:1 + dx + h]
                            nc.tensor.matmul(
                                ps[:], lt16[:, L - 1, t, :], rhs,
                                start=(t == 0), stop=(t == 8))
                    for (i0, y0), ps in zip(grp, pss):
                        psv = ps[:].rearrange(
                            "p (i h w) -> p i h w", i=ti, h=rows)
                        di = i0 if glob_dst is None else i0 + glob_dst
                        if not pool_after:
                            nc.scalar.activation(
                                xn[:, di:di + ti, 1 + y0:1 + y0 + rows,
                                   1:1 + h],
                                psv, Relu, bias=bias[:, L:L + 1])
                            continue
                        tc_t = tmp.tile([128, ti, rows, h], f32r,
                                        tag=f"tmp{h}")
                        nc.scalar.activation(
                            tc_t[:], psv, Relu, bias=bias[:, L:L + 1])
                        th = tmp.tile([128, ti, rows, h // 2], f32r,
                                      tag=f"tmph{h}")
                        t4 = tc_t[:].rearrange(
                            "p i h (w two) -> p i h w two", two=2)
                        nc.vector.tensor_max(
                            th[:], t4[:, :, :, :, 0], t4[:, :, :, :, 1])
                        t5 = th[:].rearrange(
                            "p i (h two) w -> p i h two w", two=2)
                        nc.vector.tensor_max(
                            xn[:, di:di + ti, 1 + y0 // 2:1 + (y0 + rows) // 2,
                               1:1 + h // 2],
                            t5[:, :, :, 0, :], t5[:, :, :, 1, :])

            for ch in range(N_CHUNKS):
                x0c = x0pool.tile([108, CHUNK, 32, 32], f32r, tag="x0c")
                nc.sync.dma_start(
                    out=x0c[:], in_=x0s_d[:, ch * CHUNK:(ch + 1) * CHUNK, :, :])

                # conv0: one K=108 matmul per (img, 16-row strip); all 8
                # matmuls share the same weights -> single LDWEIGHTS
                x1 = acts.tile([128, CHUNK, 34, 34], f32r, tag="big")
                nc.vector.memset(x1[:, :, 0, :].bitcast(f32), 0.0)
                nc.vector.memset(x1[:, :, 33, :].bitcast(f32), 0.0)
                nc.vector.memset(x1[:, :, 1:33, 0].bitcast(f32), 0.0)
                nc.vector.memset(x1[:, :, 1:33, 33].bitcast(f32), 0.0)
                ps0 = []
                for _pi in range(8):
                    ps_g = cps.tile([128, 512], f32, tag="cps")
                    ps0.append(ps_g)
                for i in range(CHUNK):
                    for s in range(2):
                        nc.tensor.matmul(
                            ps0[2 * i + s][:], lt0[:],
                            x0c[:, i, 16 * s:16 * s + 16, :],
                            start=True, stop=True)
                for i in range(CHUNK):
                    for s in range(2):
                        nc.scalar.activation(
                            x1[:, i, 1 + 16 * s:17 + 16 * s, 1:33],
                            ps0[2 * i + s][:].rearrange(
                                "p (h w) -> p h w", h=16),
                            Relu, bias=bias[:, 0:1])

                xk = x1
                for L in range(1, 5):
                    h = conv_h[L]
                    ti = tile_imgs[L]
                    pool_after = L in pools_after
                    hn = h // 2 if pool_after else h
                    strips = max(1, (h * h * ti) // 512)
                    rows = h // strips
                    tiles = [(it * ti, s * rows) for it in range(CHUNK // ti)
                             for s in range(strips)]
                    if L == 4:
                        xn, glob = x45, ch * CHUNK
                    else:
                        ntag = {1: "big2", 2: "med", 3: "med2"}[L]
                        xn = acts.tile([128, CHUNK, hn + 2, hn + 2], f32r,
                                       tag=ntag)
                        glob = None
                        nc.vector.memset(xn[:, :, 0, :].bitcast(f32), 0.0)
                        nc.vector.memset(xn[:, :, hn + 1, :].bitcast(f32), 0.0)
                        nc.vector.memset(xn[:, :, 1:hn + 1, 0].bitcast(f32), 0.0)
                        nc.vector.memset(
                            xn[:, :, 1:hn + 1, hn + 1].bitcast(f32), 0.0)
                    conv_layer(L, xk, xn, tiles, ti, h, rows, glob)
                    xk = xn

            # conv5 / conv6 over all 32 images per group (8-tile runs)
            for L, xk, xn in ((5, x45, x56), (6, x56, x7)):
                ti = 4
                tiles = [(it * ti, 0) for it in range(IMG_PER_GROUP // ti)]
                conv_layer(L, xk, xn, tiles, ti, 8, 8, 0 if L == 6 else None)

            # fc7: X7-stationary, K=32 row-group-g matmuls into col group 0.
            # ps7[g][i, o] = sum_{c,yx} x7[32g+c, i, yx] * w7[c, o, yx]
            f7i = persist.tile([128, 256], f32r, tag="f7i")
            ps7 = []
            for _pi in range(4):
                ps_g = cps.tile([32, 256], f32, tag="cps")
                ps7.append(ps_g)
            for t, (y, x) in enumerate((y, x) for y in range(4)
                                       for x in range(4)):
                for g in range(4):
                    nc.tensor.matmul(
                        ps7[g][:],
                        x7[32 * g:32 * g + 32, :, 1 + y, 1 + x],
                        lt7[32 * g:32 * g + 32, 4 * y + x, :],
                        start=(t == 0), stop=(t == 15),
                        tile_position=(32 * g, 0))
            tadd = persist.tile([32, 4, 256], f32, tag="tadd")
            for g in range(4):
                nc.vector.tensor_add(tadd[:, g, :], ps7[g][:], b7bc[0:32, :])
                # cross-partition write: psum-aligned rows -> sbuf rows 32g+
                nc.scalar.activation(f7i[32 * g:32 * g + 32, :],
                                     tadd[:, g, :], Relu, bias=0.0)

            # transpose -> f7t[o, img], then fc8
            f7t = persist.tile([128, 2, 128], f32r, tag="f7t")
            for hh in range(2):
                pst = cps.tile([128, 128], f32r, tag="cps")
                nc.tensor.transpose(
                    pst[:], f7i[:, 128 * hh:128 * (hh + 1)], idt[:])
                nc.scalar.activation(f7t[:, hh, :], pst[:],
                                     mybir.ActivationFunctionType.Copy)

            outt = persist.tile([10, N_IMG], f32, tag="outt")
            ps8 = cps.tile([10, N_IMG], f32, tag="cps")
            for hh in range(2):
                nc.tensor.matmul(ps8[:], lt8[:, hh, :], f7t[:, hh, :],
                                 start=(hh == 0), stop=(hh == 1))
            nc.scalar.activation(outt[:], ps8[:], Ident,
                                 bias=bias[0:10, 9:10])
            nc.sync.dma_start(out=out_d[:], in_=outt[:])

    nc.finalize()
    return nc


_NC_CACHE = None


def _get_program():
    global _NC_CACHE
    if _NC_CACHE is None:
        _NC_CACHE = _build_program()
    return _NC_CACHE


def _prep_host_inputs(x, ws, bs):
    """Build per-core input maps.  ws/bs: lists of the 9 weight/bias arrays."""
    # conv0 im2col, identical for every core: [108, 32, 32, 32]
    xp = np.zeros((N_IMG, 3, 34, 34), np.float32)
    xp[:, :, 1:33, 1:33] = x
    x0s = np.empty((108, IMG_PER_GROUP, 32, 32), np.float32)
    for g in range(4):
        sl = xp[g * 32:(g + 1) * 32]
        for c in range(3):
            for ky in range(3):
                for kx in range(3):
                    x0s[27 * g + 9 * c + 3 * ky + kx] = \
                        sl[:, c, ky:ky + 32, kx:kx + 32]
    x0s = round_fp32r(x0s)
    idt = round_fp32r(np.eye(128, dtype=np.float32))

    in_maps = []
    for m in range(N_MODELS):
        lt0 = np.zeros((108, 128), np.float32)
        w0m = ws[0][m].transpose(0, 2, 1).reshape(27, 32)  # [c,o,t]->[c,t,o]
        for g in range(4):
            lt0[27 * g:27 * g + 27, 32 * g:32 * g + 32] = w0m

        lt16 = np.zeros((128, 6, 9, 128), np.float32)
        for L in range(1, 7):
            wm = ws[L][m].transpose(0, 2, 1)  # [32c, 9t, 32o]
            for g in range(4):
                lt16[32 * g:32 * g + 32, L - 1, :, 32 * g:32 * g + 32] = wm

        # lt7[32g+c, yx, o] = w7[m, c, o, yx]  (same block for every g)
        lt7 = np.empty((128, 16, 256), np.float32)
        blk7 = ws[7][m].transpose(0, 2, 1)  # [32c, 16yx, 256o]
        for g in range(4):
            lt7[32 * g:32 * g + 32] = blk7

        lt8 = np.zeros((128, 2, 10), np.float32)
        for hh in range(2):
            lt8[:, hh, :] = ws[8][m][128 * hh:128 * (hh + 1), :, 0]

        b7bc = np.broadcast_to(bs[7][m][:, 0], (128, 256)).copy()

        biases = np.zeros((128, 10), np.float32)
        for L in range(7):
            bL = bs[L][m][:, 0]  # [32]
            for g in range(4):
                biases[32 * g:32 * g + 32, L] = bL
        biases[0:10, 9] = bs[8][m][:, 0]

        in_maps.append({
            "x0s": x0s,
            "lt0": round_fp32r(lt0),
            "lt16": round_fp32r(lt16),
            "lt7": round_fp32r(lt7),
            "lt8": round_fp32r(lt8),
            "b7bc": b7bc,
            "idt": idt,
            "biases": biases,
        })
    return in_maps


def kernel(x, w0, w1, w2, w3, w4, w5, w6, w7, w8,
           b0, b1, b2, b3, b4, b5, b6, b7, b8):
    from concourse.bass_utils import run_bass_kernel_spmd

    ws = [np.asarray(w, np.float32) for w in
          (w0, w1, w2, w3, w4, w5, w6, w7, w8)]
    bs = [np.asarray(b, np.float32) for b in
          (b0, b1, b2, b3, b4, b5, b6, b7, b8)]
    nc = _get_program()
    in_maps = _prep_host_inputs(np.asarray(x, np.float32), ws, bs)
    res = run_bass_kernel_spmd(nc, in_maps, list(range(N_MODELS)))
    out = np.stack([res.results[m]["out"].T for m in range(N_MODELS)])
    return np.ascontiguousarray(out, dtype=np.float32)


# revision 14
# speedup vs baseline: 1.1288x; 1.0055x over previous
"""Trainium2 Bass kernel for the 8-model batch-functional CNN.

Sharding: one hypernetwork model per NeuronCore (8 models / 8 cores).
Each core runs the full 7-conv + 2-fc stack for its model over all 128
images, activations resident in SBUF.

Layout: SBUF activations are [128 partitions = 4 image-groups x 32
channels, imgs, H+2, W+2] (zero-padded borders).  3x3 convs are 9
PSUM-accumulated fp32r matmuls with block-diagonal [128,128] weights
and dy/dx-shifted rhs views; matmuls are grouped taps-outer over up to
8 PSUM banks so walrus's weight-tile cache (ldw-opt) elides redundant
LDWEIGHTS.  conv0 uses a host-side im2col (K=108).  MaxPool2d(2) is
two strided tensor_max ops.  The 8x8 layers (conv5/6) run once over
all 32 images per group for longer same-weight runs.  fc7 runs
X7-stationary on the diagonal 32x32 tile positions (4 concurrent
tiles), fc8 after a PE transpose.  Image index = g*32 + i.
"""
import sys

sys.path.insert(0, "/opt/trn_rl_repo")
import numpy as np

N_MODELS = 8
N_IMG = 128
IMG_PER_GROUP = 32   # images assigned to each of the 4 partition groups
CHUNK = 4            # images per group per chunk through conv0..conv4
N_CHUNKS = IMG_PER_GROUP // CHUNK


def round_fp32r(a):
    a = np.ascontiguousarray(a, dtype=np.float32)
    b = a.view(np.uint32)
    low = b & np.uint32(0xFFF)
    bit12 = (b >> np.uint32(12)) & np.uint32(1)
    up = (low > 0x800) | ((low == 0x800) & (bit12 == 1))
    out = (b & np.uint32(0xFFFFF000)) + (up.astype(np.uint32) << np.uint32(12))
    return out.view(np.float32)


def _enable_ldw_opt():
    """Turn on walrus's weight-tile cache so back-to-back matmuls with the
    same stationary operand skip the redundant LDWEIGHTS."""
    import concourse.bass_utils as bu
    if getattr(bu.run_command, "_ldw_patched", False):
        return
    orig = bu.run_command

    def run_command(cmd, *a, **kw):
        cmd = ["--enable-ldw-opt=true" if c == "--enable-ldw-opt=false" else c
               for c in cmd]
        return orig(cmd, *a, **kw)

    run_command._ldw_patched = True
    bu.run_command = run_command


def _build_program():
    import concourse.bacc as bacc
    import concourse.tile as tile
    from concourse import mybir

    _enable_ldw_opt()

    f32 = mybir.dt.float32
    f32r = mybir.dt.float32r
    Relu = mybir.ActivationFunctionType.Relu
    Ident = mybir.ActivationFunctionType.Identity

    nc = bacc.Bacc("TRN2", target_bir_lowering=False, debug=False)

    x0s_d = nc.declare_dram_parameter("x0s", [108, IMG_PER_GROUP, 32, 32], f32r, isOutput=False)
    lt0_d = nc.declare_dram_parameter("lt0", [108, 128], f32r, isOutput=False)
    lt16_d = nc.declare_dram_parameter("lt16", [128, 6, 9, 128], f32r, isOutput=False)
    lt7_d = nc.declare_dram_parameter("lt7", [128, 16, 256], f32r, isOutput=False)
    lt8_d = nc.declare_dram_parameter("lt8", [128, 2, 10], f32r, isOutput=False)
    b7bc_d = nc.declare_dram_parameter("b7bc", [128, 256], f32, isOutput=False)
    idt_d = nc.declare_dram_parameter("idt", [128, 128], f32r, isOutput=False)
    bias_d = nc.declare_dram_parameter("biases", [128, 10], f32, isOutput=False)
    out_d = nc.declare_dram_parameter("out", [10, N_IMG], f32, isOutput=True)

    conv_h = {1: 32, 2: 32, 3: 16, 4: 16, 5: 8, 6: 8}
    pools_after = {2, 4, 6}
    tile_imgs = {1: 1, 2: 1, 3: 1, 4: 1, 5: 4, 6: 4}
    TAPS = [(dy, dx) for dy in (-1, 0, 1) for dx in (-1, 0, 1)]

    with tile.TileContext(nc) as tc:
        with tc.tile_pool(name="wpool", bufs=1) as wpool, \
             tc.tile_pool(name="acts", bufs=1) as acts, \
             tc.tile_pool(name="x0pool", bufs=2) as x0pool, \
             tc.tile_pool(name="tmp", bufs=2) as tmp, \
             tc.tile_pool(name="persist", bufs=1) as persist, \
             tc.tile_pool(name="cps", bufs=8, space="PSUM") as cps:

            lt0 = wpool.tile([108, 128], f32r, tag="lt0")
            lt16 = wpool.tile([128, 6, 9, 128], f32r, tag="lt16")
            lt7 = wpool.tile([128, 16, 256], f32r, tag="lt7")
            lt8 = wpool.tile([128, 2, 10], f32r, tag="lt8")
            b7bc = wpool.tile([128, 256], f32, tag="b7bc")
            idt = wpool.tile([128, 128], f32r, tag="idt")
            bias = wpool.tile([128, 10], f32, tag="bias")
            nc.sync.dma_start(out=lt0[:], in_=lt0_d[:])
            nc.sync.dma_start(out=bias[:], in_=bias_d[:])
            nc.gpsimd.dma_start(out=lt16[:], in_=lt16_d[:])
            nc.scalar.dma_start(out=lt7[:], in_=lt7_d[:])
            nc.scalar.dma_start(out=lt8[:], in_=lt8_d[:])
            nc.scalar.dma_start(out=b7bc[:], in_=b7bc_d[:])
            nc.scalar.dma_start(out=idt[:], in_=idt_d[:])

            # persistent buffers (across chunks)
            x45 = persist.tile([128, IMG_PER_GROUP, 10, 10], f32r, tag="x45")
            x56 = persist.tile([128, IMG_PER_GROUP, 10, 10], f32r, tag="x56")
            x7 = persist.tile([128, IMG_PER_GROUP, 6, 6], f32r, tag="x7")
            for t_ in (x45, x56, x7):
                nc.vector.memset(t_[:].bitcast(f32), 0.0)

            def conv_layer(L, xk, xn, tiles, ti, h, rows, glob_dst):
                """One 3x3 conv (+optional pool) on tiles [(i0_src, y0)].
                glob_dst: None -> xn indexed like xk; else offset added to
                i0 for the destination (pool target is a persistent buf)."""
                pool_after = L in pools_after
                nfree = ti * rows * h
                for g0 in range(0, len(tiles), 8):
                    grp = tiles[g0:g0 + 8]
                    pss = []
                    for _pi in range(len(grp)):
                        ps_g = cps.tile([128, nfree], f32, tag="cps")
                        pss.append(ps_g)
                    for t, (dy, dx) in enumerate(TAPS):
                        for (i0, y0), ps in zip(grp, pss):
                            rhs = xk[:, i0:i0 + ti,
                                     1 + y0 + dy:1 + y0 + dy + rows,
                                     1 + dx:1 + dx + h]
                            nc.tensor.matmul(
                                ps[:], lt16[:, L - 1, t, :], rhs,
                                start=(t == 0), stop=(t == 8))
                    for (i0, y0), ps in zip(grp, pss):
                        psv = ps[:].rearrange(
                            "p (i h w) -> p i h w", i=ti, h=rows)
                        di = i0 if glob_dst is None else i0 + glob_dst
                        if not pool_after:
                            nc.scalar.activation(
                                xn[:, di:di + ti, 1 + y0:1 + y0 + rows,
                                   1:1 + h],
                                psv, Relu, bias=bias[:, L:L + 1])
                            continue
                        tc_t = tmp.tile([128, ti, rows, h], f32r,
                                        tag=f"tmp{h}")
                        nc.scalar.activation(
                            tc_t[:], psv, Relu, bias=bias[:, L:L + 1])
                        th = tmp.tile([128, ti, rows, h // 2], f32r,
                                      tag=f"tmph{h}")
                        t4 = tc_t[:].rearrange(
                            "p i h (w two) -> p i h w two", two=2)
                        nc.vector.tensor_max(
                            th[:], t4[:, :, :, :, 0], t4[:, :, :, :, 1])
                        t5 = th[:].rearrange(
                            "p i (h two) w -> p i h two w", two=2)
                        nc.vector.tensor_max(
                            xn[:, di:di + ti, 1 + y0 // 2:1 + (y0 + rows) // 2,
                               1:1 + h // 2],
                            t5[:, :, :, 0, :], t5[:, :, :, 1, :])

            for ch in range(N_CHUNKS):
                x0c = x0pool.tile([108, CHUNK, 32, 32], f32r, tag="x0c")
                for i_ in range(CHUNK):
                    eng = (nc.sync, nc.gpsimd, nc.scalar, nc.sync)[i_ % 4]
                    eng.dma_start(
                        out=x0c[:, i_, :, :],
                        in_=x0s_d[:, ch * CHUNK + i_, :, :])

                # conv0: one K=108 matmul per (img, 16-row strip); all 8
                # matmuls share the same weights -> single LDWEIGHTS
                x1 = acts.tile([128, CHUNK, 34, 34], f32r, tag="big")
                nc.vector.memset(x1[:, :, 0, :].bitcast(f32), 0.0)
                nc.vector.memset(x1[:, :, 33, :].bitcast(f32), 0.0)
                nc.vector.memset(x1[:, :, 1:33, 0].bitcast(f32), 0.0)
                nc.vector.memset(x1[:, :, 1:33, 33].bitcast(f32), 0.0)
                ps0 = []
                for _pi in range(8):
                    ps_g = cps.tile([128, 512], f32, tag="cps")
                    ps0.append(ps_g)
                for i in range(CHUNK):
                    for s in range(2):
                        nc.tensor.matmul(
                            ps0[2 * i + s][:], lt0[:],
                            x0c[:, i, 16 * s:16 * s + 16, :],
                            start=True, stop=True)
                for i in range(CHUNK):
                    for s in range(2):
                        nc.scalar.activation(
                            x1[:, i, 1 + 16 * s:17 + 16 * s, 1:33],
                            ps0[2 * i + s][:].rearrange(
                                "p (h w) -> p h w", h=16),
                            Relu, bias=bias[:, 0:1])

                xk = x1
                for L in range(1, 5):
                    h = conv_h[L]
                    ti = tile_imgs[L]
                    pool_after = L in pools_after
                    hn = h // 2 if pool_after else h
                    strips = max(1, (h * h * ti) // 512)
                    rows = h // strips
                    tiles = [(it * ti, s * rows) for it in range(CHUNK // ti)
                             for s in range(strips)]
                    if L == 4:
                        xn, glob = x45, ch * CHUNK
                    else:
                        ntag = {1: "big2", 2: "med", 3: "med2"}[L]
                        xn = acts.tile([128, CHUNK, hn + 2, hn + 2], f32r,
                                       tag=ntag)
                        glob = None
                        nc.vector.memset(xn[:, :, 0, :].bitcast(f32), 0.0)
                        nc.vector.memset(xn[:, :, hn + 1, :].bitcast(f32), 0.0)
                        nc.vector.memset(xn[:, :, 1:hn + 1, 0].bitcast(f32), 0.0)
                        nc.vector.memset(
                            xn[:, :, 1:hn + 1, hn + 1].bitcast(f32), 0.0)
                    conv_layer(L, xk, xn, tiles, ti, h, rows, glob)
                    xk = xn

            # conv5 / conv6 over all 32 images per group (8-tile runs)
            for L, xk, xn in ((5, x45, x56), (6, x56, x7)):
                ti = 4
                tiles = [(it * ti, 0) for it in range(IMG_PER_GROUP // ti)]
                conv_layer(L, xk, xn, tiles, ti, 8, 8, 0 if L == 6 else None)

            # fc7: X7-stationary, K=32 row-group-g matmuls into col group 0.
            # ps7[g][i, o] = sum_{c,yx} x7[32g+c, i, yx] * w7[c, o, yx]
            f7i = persist.tile([128, 256], f32r, tag="f7i")
            ps7 = []
            for _pi in range(4):
                ps_g = cps.tile([32, 256], f32, tag="cps")
                ps7.append(ps_g)
            for t, (y, x) in enumerate((y, x) for y in range(4)
                                       for x in range(4)):
                for g in range(4):
                    nc.tensor.matmul(
                        ps7[g][:],
                        x7[32 * g:32 * g + 32, :, 1 + y, 1 + x],
                        lt7[32 * g:32 * g + 32, 4 * y + x, :],
                        start=(t == 0), stop=(t == 15),
                        tile_position=(32 * g, 0))
            tadd = persist.tile([32, 4, 256], f32, tag="tadd")
            for g in range(4):
                nc.vector.tensor_add(tadd[:, g, :], ps7[g][:], b7bc[0:32, :])
                # cross-partition write: psum-aligned rows -> sbuf rows 32g+
                nc.scalar.activation(f7i[32 * g:32 * g + 32, :],
                                     tadd[:, g, :], Relu, bias=0.0)

            # transpose -> f7t[o, img], then fc8
            f7t = persist.tile([128, 2, 128], f32r, tag="f7t")
            for hh in range(2):
                pst = cps.tile([128, 128], f32r, tag="cps")
                nc.tensor.transpose(
                    pst[:], f7i[:, 128 * hh:128 * (hh + 1)], idt[:])
                nc.scalar.activation(f7t[:, hh, :], pst[:],
                                     mybir.ActivationFunctionType.Copy)

            outt = persist.tile([10, N_IMG], f32, tag="outt")
            ps8 = cps.tile([10, N_IMG], f32, tag="cps")
            for hh in range(2):
                nc.tensor.matmul(ps8[:], lt8[:, hh, :], f7t[:, hh, :],
                                 start=(hh == 0), stop=(hh == 1))
            nc.scalar.activation(outt[:], ps8[:], Ident,
                                 bias=bias[0:10, 9:10])
            nc.sync.dma_start(out=out_d[:], in_=outt[:])

    nc.finalize()
    return nc


_NC_CACHE = None


def _get_program():
    global _NC_CACHE
    if _NC_CACHE is None:
        _NC_CACHE = _build_program()
    return _NC_CACHE


def _prep_host_inputs(x, ws, bs):
    """Build per-core input maps.  ws/bs: lists of the 9 weight/bias arrays."""
    # conv0 im2col, identical for every core: [108, 32, 32, 32]
    xp = np.zeros((N_IMG, 3, 34, 34), np.float32)
    xp[:, :, 1:33, 1:33] = x
    x0s = np.empty((108, IMG_PER_GROUP, 32, 32), np.float32)
    for g in range(4):
        sl = xp[g * 32:(g + 1) * 32]
        for c in range(3):
            for ky in range(3):
                for kx in range(3):
                    x0s[27 * g + 9 * c + 3 * ky + kx] = \
                        sl[:, c, ky:ky + 32, kx:kx + 32]
    x0s = round_fp32r(x0s)
    idt = round_fp32r(np.eye(128, dtype=np.float32))

    in_maps = []
    for m in range(N_MODELS):
        lt0 = np.zeros((108, 128), np.float32)
        w0m = ws[0][m].transpose(0, 2, 1).reshape(27, 32)  # [c,o,t]->[c,t,o]
        for g in range(4):
            lt0[27 * g:27 * g + 27, 32 * g:32 * g + 32] = w0m

        lt16 = np.zeros((128, 6, 9, 128), np.float32)
        for L in range(1, 7):
            wm = ws[L][m].transpose(0, 2, 1)  # [32c, 9t, 32o]
            for g in range(4):
                lt16[32 * g:32 * g + 32, L - 1, :, 32 * g:32 * g + 32] = wm

        # lt7[32g+c, yx, o] = w7[m, c, o, yx]  (same block for every g)
        lt7 = np.empty((128, 16, 256), np.float32)
        blk7 = ws[7][m].transpose(0, 2, 1)  # [32c, 16yx, 256o]
        for g in range(4):
            lt7[32 * g:32 * g + 32] = blk7

        lt8 = np.zeros((128, 2, 10), np.float32)
        for hh in range(2):
            lt8[:, hh, :] = ws[8][m][128 * hh:128 * (hh + 1), :, 0]

        b7bc = np.broadcast_to(bs[7][m][:, 0], (128, 256)).copy()

        biases = np.zeros((128, 10), np.float32)
        for L in range(7):
            bL = bs[L][m][:, 0]  # [32]
            for g in range(4):
                biases[32 * g:32 * g + 32, L] = bL
        biases[0:10, 9] = bs[8][m][:, 0]

        in_maps.append({
            "x0s": x0s,
            "lt0": round_fp32r(lt0),
            "lt16": round_fp32r(lt16),
            "lt7": round_fp32r(lt7),
            "lt8": round_fp32r(lt8),
            "b7bc": b7bc,
            "idt": idt,
            "biases": biases,
        })
    return in_maps


def kernel(x, w0, w1, w2, w3, w4, w5, w6, w7, w8,
           b0, b1, b2, b3, b4, b5, b6, b7, b8):
    from concourse.bass_utils import run_bass_kernel_spmd

    ws = [np.asarray(w, np.float32) for w in
          (w0, w1, w2, w3, w4, w5, w6, w7, w8)]
    bs = [np.asarray(b, np.float32) for b in
          (b0, b1, b2, b3, b4, b5, b6, b7, b8)]
    nc = _get_program()
    in_maps = _prep_host_inputs(np.asarray(x, np.float32), ws, bs)
    res = run_bass_kernel_spmd(nc, in_maps, list(range(N_MODELS)))
    out = np.stack([res.results[m]["out"].T for m in range(N_MODELS)])
    return np.ascontiguousarray(out, dtype=np.float32)
